# revision 1
# baseline (speedup 1.0000x reference)
"""DirSageConv Trainium2 kernel (8 NeuronCores, SPMD) - v4 group accumulators.

Like v3 (target-node-range edge sharding, batched int16 dma_gather,
one-hot-matmul segment sums in PSUM, fused per-block output GEMM), but:
- Blocks are accumulated per GROUP of 4 (512 nodes) in one [65, 512]
  PSUM tile: a single matmul per 128-edge tile with a [128, 512] one-hot
  covers all 4 blocks, so edges only pad at (group, chunk) granularity
  (~4x less padding than per-(block, chunk)).
- Messages carry an appended ones column (persistent bf16 slots), so the
  matmul also accumulates per-node degree counts in row 64.
- Count row is transposed per block first, then max/reciprocal run as a
  fast [128, 1] op.
Host only reorders/shards edges, converts dtypes, concatenates outputs.
"""
import numpy as np

import concourse.bass as bass
import concourse.bacc as bacc
import concourse.tile as tile
import concourse.mybir as mybir
from concourse.masks import make_identity
from concourse.library_config import mlp
from concourse.bass_utils import run_bass_kernel_spmd

P = 128
NCORES = 8
ALPHA = 0.5
CHUNK = 25000
GRP = 3
GN = GRP * P  # nodes per group
MAXC = 1024
NSLOT = 24

last_exec_time_ns = None


def _schedule(tgt, src_other, n_per_core, nblk, nchunk):
    """Per-core edge order + per-(group, chunk) padded lengths."""
    ngrp = (nblk + GRP - 1) // GRP
    cores = []
    counts = np.zeros((NCORES, ngrp, nchunk), np.int64)
    for k in range(NCORES):
        sel = np.flatnonzero((tgt >= k * n_per_core) & (tgt < (k + 1) * n_per_core))
        loc = (tgt[sel] - k * n_per_core).astype(np.int64)
        gid = src_other[sel].astype(np.int64)
        order = np.lexsort((gid // CHUNK, loc // GN))
        loc = loc[order]
        gid = gid[order]
        np.add.at(counts[k], (loc // GN, gid // CHUNK), 1)
        cores.append((gid, loc))
    L = (np.ceil(counts.max(axis=0) / P) * P).astype(np.int64)  # [ngrp, nchunk]
    empty = L.sum(axis=1) == 0
    L[empty, 0] = P
    return L, cores


def _col_of(L):
    ngrp, nchunk = L.shape
    col_of = np.zeros((ngrp, nchunk), np.int64)
    col = 0
    for g in range(ngrp):
        for c in range(nchunk):
            col_of[g, c] = col
            col += int(L[g, c]) // P
    return col_of, col


def _fill_dir(L, cores, nchunk):
    col_of, T_total = _col_of(L)
    ngrp = L.shape[0]
    gi = [np.zeros(T_total * P, np.int16) for _ in range(NCORES)]
    dr = [np.full(T_total * P, 999.0, np.float32) for _ in range(NCORES)]
    for k in range(NCORES):
        gid, loc = cores[k]
        key = (loc // GN) * nchunk + gid // CHUNK
        for g in range(ngrp):
            for c in range(nchunk):
                if L[g, c] == 0:
                    continue
                s = np.searchsorted(key, g * nchunk + c)
                e = np.searchsorted(key, g * nchunk + c, side="right")
                n = e - s
                assert n <= L[g, c]
                base = int(col_of[g, c]) * P
                gi[k][base:base + n] = (gid[s:e] - c * CHUNK).astype(np.int16)
                dr[k][base:base + n] = (loc[s:e] - g * GN).astype(np.float32)
    dr = [a.reshape(T_total, P).T.copy() for a in dr]
    return gi, dr, col_of, T_total


def _call_list(L):
    """[(g, c, nidx, wofs, colstart)]; (g,c) buckets split at MAXC."""
    ngrp, nchunk = L.shape
    calls = []
    wofs = 0
    col = 0
    for g in range(ngrp):
        for c in range(nchunk):
            nidx = int(L[g, c])
            if nidx == 0:
                continue
            for off in range(0, nidx, MAXC):
                n = min(MAXC, nidx - off)
                calls.append((g, c, n, wofs, col))
                wofs += n // 16
                col += n // P
    return calls, wofs


def _wrap_idx(gi_flat, calls):
    bufs = []
    for (g, c, nidx, wofs, col) in calls:
        seg = gi_flat[col * P: col * P + nidx]
        w = seg.reshape(nidx // 16, 16).T
        bufs.append(np.tile(w, (8, 1)))
    return np.ascontiguousarray(np.concatenate(bufs, axis=1))


def _build_program(N, D, nblk, npad, Lin, Lout, calls_in, calls_out,
                   colof_in, colof_out, T_in, T_out, W_in, W_out):
    nc = bacc.Bacc("TRN2", target_bir_lowering=False, debug=False,
                   num_devices=NCORES)
    f32 = mybir.dt.float32
    bf16 = mybir.dt.bfloat16
    ngrp, nchunk = Lin.shape
    xg_d = nc.dram_tensor("xg", [N, D], f32, kind="ExternalInput")
    xt_d = nc.dram_tensor("xt", [D, npad], bf16, kind="ExternalInput")
    gii_d = nc.dram_tensor("gii", [P, W_in], mybir.dt.int16, kind="ExternalInput")
    gio_d = nc.dram_tensor("gio", [P, W_out], mybir.dt.int16, kind="ExternalInput")
    dri_d = nc.dram_tensor("dri", [P, T_in], f32, kind="ExternalInput")
    dro_d = nc.dram_tensor("dro", [P, T_out], f32, kind="ExternalInput")
    w1_d = nc.dram_tensor("w1", [D, D], bf16, kind="ExternalInput")
    w2_d = nc.dram_tensor("w2", [D, D], bf16, kind="ExternalInput")
    w3_d = nc.dram_tensor("w3", [D, D], bf16, kind="ExternalInput")
    bb_d = nc.dram_tensor("bb", [1, D], bf16, kind="ExternalInput")
    out_d = nc.dram_tensor("out", [npad, D], f32, kind="ExternalOutput")

    def gtiles(L, g):
        return int(sum(L[g, c] for c in range(L.shape[1]))) // P

    maxTG = max(max(gtiles(Lin, g) for g in range(ngrp)),
                max(gtiles(Lout, g) for g in range(ngrp)))

    with tile.TileContext(nc) as tc:
        with (
            tc.tile_pool(name="const", bufs=1) as cpool,
            tc.tile_pool(name="dest", bufs=2) as dpool,
            tc.tile_pool(name="mslots", bufs=1) as mpool,
            tc.tile_pool(name="hpool", bufs=8) as hpool,
            tc.tile_pool(name="fin", bufs=3) as fpool,
            tc.tile_pool(name="pbi", bufs=2, space="PSUM") as pbi,
            tc.tile_pool(name="pbo", bufs=2, space="PSUM") as pbo,
            tc.tile_pool(name="tpp", bufs=1, space="PSUM") as tpp,
            tc.tile_pool(name="opp", bufs=1, space="PSUM") as opp,
            tc.tile_pool(name="app", bufs=1, space="PSUM") as app,
        ):
            nc.gpsimd.load_library(mlp)
            gii_sb = cpool.tile([P, W_in], mybir.dt.int16, name="gii_sb")
            nc.sync.dma_start(out=gii_sb[:], in_=gii_d[:])
            gio_sb = cpool.tile([P, W_out], mybir.dt.int16, name="gio_sb")
            nc.sync.dma_start(out=gio_sb[:], in_=gio_d[:])
            dri_sb = cpool.tile([P, T_in], f32, name="dri_sb")
            nc.sync.dma_start(out=dri_sb[:], in_=dri_d[:])
            dro_sb = cpool.tile([P, T_out], f32, name="dro_sb")
            nc.sync.dma_start(out=dro_sb[:], in_=dro_d[:])
            xt_sb = cpool.tile([D, npad], bf16, name="xt_sb")
            nc.sync.dma_start(out=xt_sb[:], in_=xt_d[:])
            w1_sb = cpool.tile([D, D], bf16, name="w1_sb")
            nc.sync.dma_start(out=w1_sb[:], in_=w1_d[:])
            w2_sb = cpool.tile([D, D], bf16, name="w2_sb")
            nc.sync.dma_start(out=w2_sb[:], in_=w2_d[:])
            w3_sb = cpool.tile([D, D], bf16, name="w3_sb")
            nc.sync.dma_start(out=w3_sb[:], in_=w3_d[:])
            bb_sb = cpool.tile([1, D], bf16, name="bb_sb")
            nc.sync.dma_start(out=bb_sb[:], in_=bb_d[:])
            ident = cpool.tile([P, P], f32, name="ident")
            make_identity(nc, ident[:])
            iota_t = cpool.tile([P, GN], f32, name="iota_t")
            nc.gpsimd.iota(iota_t[:], pattern=[[1, GN]], base=0,
                           channel_multiplier=0,
                           allow_small_or_imprecise_dtypes=True)
            ones1 = cpool.tile([1, P], bf16, name="ones1")
            nc.vector.memset(ones1[:], 1.0)

            m_slots = []
            for i in range(NSLOT):
                m = mpool.tile([P, D + 1], bf16, name=f"mslot{i}",
                               tag=f"mslot{i}")
                nc.vector.memset(m[:, D:D + 1], 1.0)
                m_slots.append(m)

            tcnt = {"all": 0}

            def do_group(g, dn, gi_sb, dr_sb, L, colof, calls, pool):
                ntg = gtiles(L, g)
                start_col = int(colof[g, 0])
                dest = dpool.tile([P, maxTG, D], f32, tag=f"dest_{dn}",
                                  name=f"dest_{dn}{g}")

                for (gg, c, nidx, wofs, colstart) in calls:
                    if gg != g:
                        continue
                    rel = colstart - start_col
                    ncols = nidx // P
                    nc.gpsimd.dma_gather(
                        out_ap=dest[:, rel:rel + ncols, :],
                        in_ap=xg_d[c * CHUNK:min((c + 1) * CHUNK, N), :],
                        idxs_ap=gi_sb[:, wofs:wofs + nidx // 16],
                        num_idxs=nidx,
                        num_idxs_reg=nidx,
                        elem_size=D,
                        single_packet=False,
                    )
                pbTg = pool.tile([D + 1, GN], f32, tag="pb", name=f"pb_{dn}{g}")
                for t in range(ntg):
                    col = start_col + t
                    m = m_slots[tcnt["all"] % NSLOT]
                    nc.any.tensor_copy(out=m[:, 0:D], in_=dest[:, t, :])
                    h = hpool.tile([P, GN], bf16, tag=f"h_{dn}",
                                   name=f"h_{dn}{col}")
                    nc.vector.tensor_tensor(
                        out=h[:],
                        in0=dr_sb[:, col:col + 1].to_broadcast([P, GN]),
                        in1=iota_t[:],
                        op=mybir.AluOpType.is_equal)
                    nc.tensor.matmul(pbTg[:], lhsT=m[:], rhs=h[:],
                                     start=(t == 0), stop=(t == ntg - 1))
                    tcnt["all"] += 1
                aggTg = fpool.tile([D, GN], bf16, tag=f"aggTg_{dn}",
                                   name=f"aggTg_{dn}{g}")
                nc.vector.tensor_copy(out=aggTg[:], in_=pbTg[0:D, :])
                cntr = fpool.tile([1, GN], f32, tag=f"cntr_{dn}",
                                  name=f"cntr_{dn}{g}")
                nc.vector.tensor_copy(out=cntr[:], in_=pbTg[D:D + 1, :])
                return aggTg, cntr

            ngrp_blocks = [min(GRP, nblk - g * GRP) for g in range(ngrp)]

            for g in range(ngrp):
                aggTg_in, cntr_in = do_group(g, "in", gii_sb, dri_sb, Lin,
                                             colof_in, calls_in, pbi)
                aggTg_out, cntr_out = do_group(g, "out", gio_sb, dro_sb, Lout,
                                               colof_out, calls_out, pbo)
                for j in range(ngrp_blocks[g]):
                    b = g * GRP + j
                    invs = []
                    for dn, cntr in (("i", cntr_in), ("o", cntr_out)):
                        tps = tpp.tile([P, 1], f32, tag="tps",
                                       name=f"tps_{dn}{b}")
                        nc.tensor.transpose(
                            out=tps[:], in_=cntr[0:1, j * P:(j + 1) * P],
                            identity=ident[0:1, 0:1])
                        invc = fpool.tile([P, 1], f32, tag=f"invc_{dn}",
                                          name=f"invc_{dn}{b}")
                        nc.vector.tensor_scalar_max(out=invc[:], in0=tps[:],
                                                    scalar1=1.0)
                        nc.vector.reciprocal(out=invc[:], in_=invc[:])
                        invs.append(invc)
                    inv_in, inv_out = invs
                    ops = opp.tile([P, D], f32, tag="ops", name=f"ops{b}")
                    nc.tensor.matmul(ops[:], lhsT=xt_sb[:, b * P:(b + 1) * P],
                                     rhs=w1_sb[:], start=True, stop=False)
                    nc.tensor.matmul(ops[:], lhsT=ones1[:], rhs=bb_sb[:],
                                     start=False, stop=True)
                    aps_in = app.tile([P, D], f32, tag="apsi", name=f"apsi{b}")
                    nc.tensor.matmul(aps_in[:],
                                     lhsT=aggTg_in[:, j * P:(j + 1) * P],
                                     rhs=w2_sb[:], start=True, stop=True)
                    aps_out = app.tile([P, D], f32, tag="apso",
                                       name=f"apso{b}")
                    nc.tensor.matmul(aps_out[:],
                                     lhsT=aggTg_out[:, j * P:(j + 1) * P],
                                     rhs=w3_sb[:], start=True, stop=True)
                    c1 = fpool.tile([P, D], f32, tag="c1", name=f"c1{b}")
                    nc.vector.tensor_tensor(
                        out=c1[:], in0=aps_in[:],
                        in1=inv_in[:].to_broadcast([P, D]),
                        op=mybir.AluOpType.mult)
                    c2 = fpool.tile([P, D], f32, tag="c2", name=f"c2{b}")
                    nc.vector.tensor_tensor(
                        out=c2[:], in0=aps_out[:],
                        in1=inv_out[:].to_broadcast([P, D]),
                        op=mybir.AluOpType.mult)
                    nc.vector.tensor_tensor(out=c1[:], in0=c1[:], in1=c2[:],
                                            op=mybir.AluOpType.add)
                    osb = fpool.tile([P, D], f32, tag="osb", name=f"osb{b}")
                    nc.vector.tensor_tensor(out=osb[:], in0=c1[:], in1=ops[:],
                                            op=mybir.AluOpType.add)
                    nc.sync.dma_start(out=out_d[b * P:(b + 1) * P, :],
                                      in_=osb[:])

    nc.compile()
    return nc


def kernel(x, edge_index, W_s2d, W_d2s, W_self, b_self):
    global last_exec_time_ns
    import ml_dtypes
    bf = ml_dtypes.bfloat16
    N, D = x.shape
    assert N % NCORES == 0
    n_per_core = N // NCORES
    nblk = (n_per_core + P - 1) // P
    npad = nblk * P
    nchunk = (N + CHUNK - 1) // CHUNK
    src = np.asarray(edge_index[0], dtype=np.int64)
    dst = np.asarray(edge_index[1], dtype=np.int64)

    Lin, cores_in = _schedule(dst, src, n_per_core, nblk, nchunk)
    Lout, cores_out = _schedule(src, dst, n_per_core, nblk, nchunk)
    gi_in, dr_in, colof_in, T_in = _fill_dir(Lin, cores_in, nchunk)
    gi_out, dr_out, colof_out, T_out = _fill_dir(Lout, cores_out, nchunk)
    calls_in, W_in = _call_list(Lin)
    calls_out, W_out = _call_list(Lout)
    gii_list = [_wrap_idx(gi_in[k], calls_in) for k in range(NCORES)]
    gio_list = [_wrap_idx(gi_out[k], calls_out) for k in range(NCORES)]

    x32 = np.ascontiguousarray(x, dtype=np.float32)
    xt = np.zeros((NCORES, D, npad), bf)
    for k in range(NCORES):
        xt[k, :, :n_per_core] = \
            x32[k * n_per_core:(k + 1) * n_per_core].T.astype(bf)

    w1 = np.ascontiguousarray(np.asarray(W_self, np.float32).astype(bf))
    w2 = np.ascontiguousarray(
        ((1.0 - ALPHA) * np.asarray(W_s2d, np.float32)).astype(bf))
    w3 = np.ascontiguousarray(
        (ALPHA * np.asarray(W_d2s, np.float32)).astype(bf))
    bb = np.asarray(b_self, np.float32).reshape(1, D).astype(bf)

    nc = _build_program(N, D, nblk, npad, Lin, Lout, calls_in, calls_out,
                        colof_in, colof_out, T_in, T_out, W_in, W_out)
    in_maps = [
        {"xg": x32, "xt": np.ascontiguousarray(xt[k]),
         "gii": gii_list[k], "gio": gio_list[k],
         "dri": dr_in[k], "dro": dr_out[k],
         "w1": w1, "w2": w2, "w3": w3, "bb": bb}
        for k in range(NCORES)
    ]
    res = run_bass_kernel_spmd(nc, in_maps, list(range(NCORES)),
                               trace=_tracing_enabled())
    last_exec_time_ns = res.exec_time_ns
    out = np.concatenate(
        [res.results[k]["out"][:n_per_core] for k in range(NCORES)], axis=0)
    return out


def _tracing_enabled():
    import os
    return os.environ.get("KERNEL_TRACE", "0") == "1"


if True:
    try:
        import contextlib
        import ctypes
        import sys
        import types

        def _mk():
            lib = ctypes.CDLL("/opt/axon/libaxon_pjrt.so")
            if not hasattr(lib, "axon_start_nrt_profile"):
                return None
            lib.axon_start_nrt_profile.argtypes = [
                ctypes.POINTER(ctypes.c_int64), ctypes.c_size_t]
            lib.axon_start_nrt_profile.restype = ctypes.c_int64
            lib.axon_stop_nrt_profile.argtypes = [ctypes.c_char_p]
            lib.axon_stop_nrt_profile.restype = ctypes.c_int64

            @contextlib.contextmanager
            def _hook(output_dir, device_ids):
                import jax
                jax.devices()
                if device_ids:
                    ids = (ctypes.c_int64 * len(device_ids))(*device_ids)
                    rc = lib.axon_start_nrt_profile(ids, len(device_ids))
                else:
                    rc = lib.axon_start_nrt_profile(None, 0)
                if rc != 0:
                    raise RuntimeError(f"start_nrt_profile rc={rc}")
                try:
                    yield
                finally:
                    n = lib.axon_stop_nrt_profile(str(output_dir).encode())
                    if n < 0:
                        raise RuntimeError(f"stop_nrt_profile rc={n}")

            return _hook

        _hk = _mk()
        _m = types.ModuleType("antenv.axon_hooks")
        _m.get_axon_ntff_profile_hook = lambda: _hk
        sys.modules["antenv.axon_hooks"] = _m
        import concourse.bass_utils as _bu
        _bu.upload_artifacts = lambda t: str(t)
    except Exception:
        pass



# revision 4
# speedup vs baseline: 2.3290x; 2.3290x over previous
"""DirSageConv Trainium2 kernel (8 NeuronCores, SPMD) - v4 group accumulators.

Like v3 (target-node-range edge sharding, batched int16 dma_gather,
one-hot-matmul segment sums in PSUM, fused per-block output GEMM), but:
- Blocks are accumulated per GROUP of 4 (512 nodes) in one [65, 512]
  PSUM tile: a single matmul per 128-edge tile with a [128, 512] one-hot
  covers all 4 blocks, so edges only pad at (group, chunk) granularity
  (~4x less padding than per-(block, chunk)).
- Messages carry an appended ones column (persistent bf16 slots), so the
  matmul also accumulates per-node degree counts in row 64.
- Count row is transposed per block first, then max/reciprocal run as a
  fast [128, 1] op.
Host only reorders/shards edges, converts dtypes, concatenates outputs.
"""
import numpy as np

import concourse.bass as bass
import concourse.bacc as bacc
import concourse.tile as tile
import concourse.mybir as mybir
from concourse.masks import make_identity
from concourse.library_config import mlp
from concourse.bass_utils import run_bass_kernel_spmd

P = 128
NCORES = 8
ALPHA = 0.5
CHUNK = 25000
GRP = 3
GN = GRP * P  # nodes per group
MAXC = 1024
NSLOT = 24

last_exec_time_ns = None


def _schedule(tgt, src_other, n_per_core, nblk, nchunk):
    """Per-core edge order + per-(group, chunk) padded lengths."""
    ngrp = (nblk + GRP - 1) // GRP
    cores = []
    counts = np.zeros((NCORES, ngrp, nchunk), np.int64)
    for k in range(NCORES):
        sel = np.flatnonzero((tgt >= k * n_per_core) & (tgt < (k + 1) * n_per_core))
        loc = (tgt[sel] - k * n_per_core).astype(np.int64)
        gid = src_other[sel].astype(np.int64)
        order = np.lexsort((gid // CHUNK, loc // GN))
        loc = loc[order]
        gid = gid[order]
        np.add.at(counts[k], (loc // GN, gid // CHUNK), 1)
        cores.append((gid, loc))
    L = (np.ceil(counts.max(axis=0) / P) * P).astype(np.int64)  # [ngrp, nchunk]
    empty = L.sum(axis=1) == 0
    L[empty, 0] = P
    return L, cores


def _col_of(L):
    ngrp, nchunk = L.shape
    col_of = np.zeros((ngrp, nchunk), np.int64)
    col = 0
    for g in range(ngrp):
        for c in range(nchunk):
            col_of[g, c] = col
            col += int(L[g, c]) // P
    return col_of, col


def _fill_dir(L, cores, nchunk):
    col_of, T_total = _col_of(L)
    ngrp = L.shape[0]
    gi = [np.zeros(T_total * P, np.int16) for _ in range(NCORES)]
    dr = [np.full(T_total * P, 999.0, np.float32) for _ in range(NCORES)]
    for k in range(NCORES):
        gid, loc = cores[k]
        key = (loc // GN) * nchunk + gid // CHUNK
        for g in range(ngrp):
            for c in range(nchunk):
                if L[g, c] == 0:
                    continue
                s = np.searchsorted(key, g * nchunk + c)
                e = np.searchsorted(key, g * nchunk + c, side="right")
                n = e - s
                assert n <= L[g, c]
                base = int(col_of[g, c]) * P
                gi[k][base:base + n] = (gid[s:e] - c * CHUNK).astype(np.int16)
                dr[k][base:base + n] = (loc[s:e] - g * GN).astype(np.float32)
    dr = [a.reshape(T_total, P).T.copy() for a in dr]
    return gi, dr, col_of, T_total


def _call_list(L):
    """[(g, c, nidx, wofs, colstart)]; (g,c) buckets split at MAXC."""
    ngrp, nchunk = L.shape
    calls = []
    wofs = 0
    col = 0
    for g in range(ngrp):
        for c in range(nchunk):
            nidx = int(L[g, c])
            if nidx == 0:
                continue
            for off in range(0, nidx, MAXC):
                n = min(MAXC, nidx - off)
                calls.append((g, c, n, wofs, col))
                wofs += n // 16
                col += n // P
    return calls, wofs


def _wrap_idx(gi_flat, calls):
    bufs = []
    for (g, c, nidx, wofs, col) in calls:
        seg = gi_flat[col * P: col * P + nidx]
        w = seg.reshape(nidx // 16, 16).T
        bufs.append(np.tile(w, (8, 1)))
    return np.ascontiguousarray(np.concatenate(bufs, axis=1))


def _build_program(N, D, nblk, npad, Lin, Lout, calls_in, calls_out,
                   colof_in, colof_out, T_in, T_out, W_in, W_out):
    nc = bacc.Bacc("TRN2", target_bir_lowering=False, debug=False,
                   num_devices=NCORES, num_swdge_queues=4)
    f32 = mybir.dt.float32
    bf16 = mybir.dt.bfloat16
    ngrp, nchunk = Lin.shape
    xg_d = nc.dram_tensor("xg", [N, D], f32, kind="ExternalInput")
    xt_d = nc.dram_tensor("xt", [D, npad], bf16, kind="ExternalInput")
    gii_d = nc.dram_tensor("gii", [P, W_in], mybir.dt.int16, kind="ExternalInput")
    gio_d = nc.dram_tensor("gio", [P, W_out], mybir.dt.int16, kind="ExternalInput")
    dri_d = nc.dram_tensor("dri", [P, T_in], f32, kind="ExternalInput")
    dro_d = nc.dram_tensor("dro", [P, T_out], f32, kind="ExternalInput")
    w1_d = nc.dram_tensor("w1", [D, D], bf16, kind="ExternalInput")
    w2_d = nc.dram_tensor("w2", [D, D], bf16, kind="ExternalInput")
    w3_d = nc.dram_tensor("w3", [D, D], bf16, kind="ExternalInput")
    bb_d = nc.dram_tensor("bb", [1, D], bf16, kind="ExternalInput")
    out_d = nc.dram_tensor("out", [npad, D], f32, kind="ExternalOutput")

    def gtiles(L, g):
        return int(sum(L[g, c] for c in range(L.shape[1]))) // P

    maxTG = max(max(gtiles(Lin, g) for g in range(ngrp)),
                max(gtiles(Lout, g) for g in range(ngrp)))

    with tile.TileContext(nc) as tc:
        with (
            tc.tile_pool(name="const", bufs=1) as cpool,
            tc.tile_pool(name="dest", bufs=2) as dpool,
            tc.tile_pool(name="mslots", bufs=1) as mpool,
            tc.tile_pool(name="hpool", bufs=8) as hpool,
            tc.tile_pool(name="fin", bufs=3) as fpool,
            tc.tile_pool(name="pbi", bufs=2, space="PSUM") as pbi,
            tc.tile_pool(name="pbo", bufs=2, space="PSUM") as pbo,
            tc.tile_pool(name="tpp", bufs=1, space="PSUM") as tpp,
            tc.tile_pool(name="opp", bufs=1, space="PSUM") as opp,
            tc.tile_pool(name="app", bufs=1, space="PSUM") as app,
        ):
            nc.gpsimd.load_library(mlp)
            gii_sb = cpool.tile([P, W_in], mybir.dt.int16, name="gii_sb")
            nc.sync.dma_start(out=gii_sb[:], in_=gii_d[:])
            gio_sb = cpool.tile([P, W_out], mybir.dt.int16, name="gio_sb")
            nc.sync.dma_start(out=gio_sb[:], in_=gio_d[:])
            dri_sb = cpool.tile([P, T_in], f32, name="dri_sb")
            nc.sync.dma_start(out=dri_sb[:], in_=dri_d[:])
            dro_sb = cpool.tile([P, T_out], f32, name="dro_sb")
            nc.sync.dma_start(out=dro_sb[:], in_=dro_d[:])
            xt_sb = cpool.tile([D, npad], bf16, name="xt_sb")
            nc.sync.dma_start(out=xt_sb[:], in_=xt_d[:])
            w1_sb = cpool.tile([D, D], bf16, name="w1_sb")
            nc.sync.dma_start(out=w1_sb[:], in_=w1_d[:])
            w2_sb = cpool.tile([D, D], bf16, name="w2_sb")
            nc.sync.dma_start(out=w2_sb[:], in_=w2_d[:])
            w3_sb = cpool.tile([D, D], bf16, name="w3_sb")
            nc.sync.dma_start(out=w3_sb[:], in_=w3_d[:])
            bb_sb = cpool.tile([1, D], bf16, name="bb_sb")
            nc.sync.dma_start(out=bb_sb[:], in_=bb_d[:])
            ident = cpool.tile([P, P], f32, name="ident")
            make_identity(nc, ident[:])
            iota_t = cpool.tile([P, GN], f32, name="iota_t")
            nc.gpsimd.iota(iota_t[:], pattern=[[1, GN]], base=0,
                           channel_multiplier=0,
                           allow_small_or_imprecise_dtypes=True)
            ones1 = cpool.tile([1, P], bf16, name="ones1")
            nc.vector.memset(ones1[:], 1.0)

            m_slots = []
            for i in range(NSLOT):
                m = mpool.tile([P, D + 1], bf16, name=f"mslot{i}",
                               tag=f"mslot{i}")
                nc.vector.memset(m[:, D:D + 1], 1.0)
                m_slots.append(m)

            tcnt = {"all": 0}
            qcnt = {"q": 0}

            def do_group(g, dn, gi_sb, dr_sb, L, colof, calls, pool):
                ntg = gtiles(L, g)
                start_col = int(colof[g, 0])
                dest = dpool.tile([P, maxTG, D], f32, tag=f"dest_{dn}",
                                  name=f"dest_{dn}{g}")

                for (gg, c, nidx, wofs, colstart) in calls:
                    if gg != g:
                        continue
                    rel = colstart - start_col
                    ncols = nidx // P
                    nc.gpsimd.dma_gather(
                        out_ap=dest[:, rel:rel + ncols, :],
                        in_ap=xg_d[c * CHUNK:min((c + 1) * CHUNK, N), :],
                        idxs_ap=gi_sb[:, wofs:wofs + nidx // 16],
                        num_idxs=nidx,
                        num_idxs_reg=nidx,
                        elem_size=D,
                        single_packet=False,
                        queue_num=qcnt["q"] % 4,
                    )
                    qcnt["q"] += 1
                pbTg = pool.tile([D + 1, GN], f32, tag="pb", name=f"pb_{dn}{g}")
                for t in range(ntg):
                    col = start_col + t
                    m = m_slots[tcnt["all"] % NSLOT]
                    nc.any.tensor_copy(out=m[:, 0:D], in_=dest[:, t, :])
                    h = hpool.tile([P, GN], bf16, tag=f"h_{dn}",
                                   name=f"h_{dn}{col}")
                    nc.vector.tensor_tensor(
                        out=h[:],
                        in0=dr_sb[:, col:col + 1].to_broadcast([P, GN]),
                        in1=iota_t[:],
                        op=mybir.AluOpType.is_equal)
                    nc.tensor.matmul(pbTg[:], lhsT=m[:], rhs=h[:],
                                     start=(t == 0), stop=(t == ntg - 1))
                    tcnt["all"] += 1
                aggTg = fpool.tile([D, GN], bf16, tag=f"aggTg_{dn}",
                                   name=f"aggTg_{dn}{g}")
                nc.vector.tensor_copy(out=aggTg[:], in_=pbTg[0:D, :])
                cntr = fpool.tile([1, GN], f32, tag=f"cntr_{dn}",
                                  name=f"cntr_{dn}{g}")
                nc.vector.tensor_copy(out=cntr[:], in_=pbTg[D:D + 1, :])
                return aggTg, cntr

            ngrp_blocks = [min(GRP, nblk - g * GRP) for g in range(ngrp)]

            for g in range(ngrp):
                aggTg_in, cntr_in = do_group(g, "in", gii_sb, dri_sb, Lin,
                                             colof_in, calls_in, pbi)
                aggTg_out, cntr_out = do_group(g, "out", gio_sb, dro_sb, Lout,
                                               colof_out, calls_out, pbo)
                for j in range(ngrp_blocks[g]):
                    b = g * GRP + j
                    invs = []
                    for dn, cntr in (("i", cntr_in), ("o", cntr_out)):
                        tps = tpp.tile([P, 1], f32, tag="tps",
                                       name=f"tps_{dn}{b}")
                        nc.tensor.transpose(
                            out=tps[:], in_=cntr[0:1, j * P:(j + 1) * P],
                            identity=ident[0:1, 0:1])
                        invc = fpool.tile([P, 1], f32, tag=f"invc_{dn}",
                                          name=f"invc_{dn}{b}")
                        nc.vector.tensor_scalar_max(out=invc[:], in0=tps[:],
                                                    scalar1=1.0)
                        nc.vector.reciprocal(out=invc[:], in_=invc[:])
                        invs.append(invc)
                    inv_in, inv_out = invs
                    ops = opp.tile([P, D], f32, tag="ops", name=f"ops{b}")
                    nc.tensor.matmul(ops[:], lhsT=xt_sb[:, b * P:(b + 1) * P],
                                     rhs=w1_sb[:], start=True, stop=False)
                    nc.tensor.matmul(ops[:], lhsT=ones1[:], rhs=bb_sb[:],
                                     start=False, stop=True)
                    aps_in = app.tile([P, D], f32, tag="apsi", name=f"apsi{b}")
                    nc.tensor.matmul(aps_in[:],
                                     lhsT=aggTg_in[:, j * P:(j + 1) * P],
                                     rhs=w2_sb[:], start=True, stop=True)
                    aps_out = app.tile([P, D], f32, tag="apso",
                                       name=f"apso{b}")
                    nc.tensor.matmul(aps_out[:],
                                     lhsT=aggTg_out[:, j * P:(j + 1) * P],
                                     rhs=w3_sb[:], start=True, stop=True)
                    c1 = fpool.tile([P, D], f32, tag="c1", name=f"c1{b}")
                    nc.vector.tensor_tensor(
                        out=c1[:], in0=aps_in[:],
                        in1=inv_in[:].to_broadcast([P, D]),
                        op=mybir.AluOpType.mult)
                    c2 = fpool.tile([P, D], f32, tag="c2", name=f"c2{b}")
                    nc.vector.tensor_tensor(
                        out=c2[:], in0=aps_out[:],
                        in1=inv_out[:].to_broadcast([P, D]),
                        op=mybir.AluOpType.mult)
                    nc.vector.tensor_tensor(out=c1[:], in0=c1[:], in1=c2[:],
                                            op=mybir.AluOpType.add)
                    osb = fpool.tile([P, D], f32, tag="osb", name=f"osb{b}")
                    nc.vector.tensor_tensor(out=osb[:], in0=c1[:], in1=ops[:],
                                            op=mybir.AluOpType.add)
                    nc.sync.dma_start(out=out_d[b * P:(b + 1) * P, :],
                                      in_=osb[:])

    nc.compile()
    return nc


def kernel(x, edge_index, W_s2d, W_d2s, W_self, b_self):
    global last_exec_time_ns
    import ml_dtypes
    bf = ml_dtypes.bfloat16
    N, D = x.shape
    assert N % NCORES == 0
    n_per_core = N // NCORES
    nblk = (n_per_core + P - 1) // P
    npad = nblk * P
    nchunk = (N + CHUNK - 1) // CHUNK
    src = np.asarray(edge_index[0], dtype=np.int64)
    dst = np.asarray(edge_index[1], dtype=np.int64)

    Lin, cores_in = _schedule(dst, src, n_per_core, nblk, nchunk)
    Lout, cores_out = _schedule(src, dst, n_per_core, nblk, nchunk)
    gi_in, dr_in, colof_in, T_in = _fill_dir(Lin, cores_in, nchunk)
    gi_out, dr_out, colof_out, T_out = _fill_dir(Lout, cores_out, nchunk)
    calls_in, W_in = _call_list(Lin)
    calls_out, W_out = _call_list(Lout)
    gii_list = [_wrap_idx(gi_in[k], calls_in) for k in range(NCORES)]
    gio_list = [_wrap_idx(gi_out[k], calls_out) for k in range(NCORES)]

    x32 = np.ascontiguousarray(x, dtype=np.float32)
    xt = np.zeros((NCORES, D, npad), bf)
    for k in range(NCORES):
        xt[k, :, :n_per_core] = \
            x32[k * n_per_core:(k + 1) * n_per_core].T.astype(bf)

    w1 = np.ascontiguousarray(np.asarray(W_self, np.float32).astype(bf))
    w2 = np.ascontiguousarray(
        ((1.0 - ALPHA) * np.asarray(W_s2d, np.float32)).astype(bf))
    w3 = np.ascontiguousarray(
        (ALPHA * np.asarray(W_d2s, np.float32)).astype(bf))
    bb = np.asarray(b_self, np.float32).reshape(1, D).astype(bf)

    nc = _build_program(N, D, nblk, npad, Lin, Lout, calls_in, calls_out,
                        colof_in, colof_out, T_in, T_out, W_in, W_out)
    in_maps = [
        {"xg": x32, "xt": np.ascontiguousarray(xt[k]),
         "gii": gii_list[k], "gio": gio_list[k],
         "dri": dr_in[k], "dro": dr_out[k],
         "w1": w1, "w2": w2, "w3": w3, "bb": bb}
        for k in range(NCORES)
    ]
    res = run_bass_kernel_spmd(nc, in_maps, list(range(NCORES)),
                               trace=_tracing_enabled())
    last_exec_time_ns = res.exec_time_ns
    out = np.concatenate(
        [res.results[k]["out"][:n_per_core] for k in range(NCORES)], axis=0)
    return out


def _tracing_enabled():
    import os
    return os.environ.get("KERNEL_TRACE", "0") == "1"


if True:
    try:
        import contextlib
        import ctypes
        import sys
        import types

        def _mk():
            lib = ctypes.CDLL("/opt/axon/libaxon_pjrt.so")
            if not hasattr(lib, "axon_start_nrt_profile"):
                return None
            lib.axon_start_nrt_profile.argtypes = [
                ctypes.POINTER(ctypes.c_int64), ctypes.c_size_t]
            lib.axon_start_nrt_profile.restype = ctypes.c_int64
            lib.axon_stop_nrt_profile.argtypes = [ctypes.c_char_p]
            lib.axon_stop_nrt_profile.restype = ctypes.c_int64

            @contextlib.contextmanager
            def _hook(output_dir, device_ids):
                import jax
                jax.devices()
                if device_ids:
                    ids = (ctypes.c_int64 * len(device_ids))(*device_ids)
                    rc = lib.axon_start_nrt_profile(ids, len(device_ids))
                else:
                    rc = lib.axon_start_nrt_profile(None, 0)
                if rc != 0:
                    raise RuntimeError(f"start_nrt_profile rc={rc}")
                try:
                    yield
                finally:
                    n = lib.axon_stop_nrt_profile(str(output_dir).encode())
                    if n < 0:
                        raise RuntimeError(f"stop_nrt_profile rc={n}")

            return _hook

        _hk = _mk()
        _m = types.ModuleType("antenv.axon_hooks")
        _m.get_axon_ntff_profile_hook = lambda: _hk
        sys.modules["antenv.axon_hooks"] = _m
        import concourse.bass_utils as _bu
        _bu.upload_artifacts = lambda t: str(t)
    except Exception:
        pass



# revision 10
# speedup vs baseline: 3.6071x; 1.5488x over previous
"""DirSageConv Trainium2 kernel (8 NeuronCores, SPMD) - v6.

Target-node-range edge sharding; per-(block 128, chunk 25000) buckets,
edges ordered (superblock, chunk, block). Per (superblock, chunk) one
dma_gather call (4 SWDGE queues round-robin, 32KB descriptor scratch),
f32 messages cast to fp16 in bulk on Scalar, fp16 one-hot built in bulk
on Vector (3-D broadcast is_equal vs iota), per-tile matmul accumulates
block sums into a per-superblock PSUM strip [64, S*128]. Mean division
uses host-precomputed reciprocal in/out degrees. Final per-block GEMMs
(fp16) add self path + bias.
"""
import numpy as np

import concourse.bass as bass
import concourse.bacc as bacc
import concourse.tile as tile
import concourse.mybir as mybir
from concourse.library_config import mlp
from concourse.bass_utils import run_bass_kernel_spmd

import os

P = 128
NCORES = 8
ALPHA = 0.5
CHUNK = 25000
SB = 4          # blocks per superblock
MAXC = 2048     # max idxs per dma_gather call (= scratch/16)
SCRATCH = 32768
NQ = 1 if os.environ.get("KERNEL_FORCE_Q0") else 4

last_exec_time_ns = None


def _schedule(tgt, src_other, n_per_core, nblk, nchunk):
    """Per-core edge order + per-(block, chunk) padded lengths."""
    cores = []
    counts = np.zeros((NCORES, nblk, nchunk), np.int64)
    for k in range(NCORES):
        sel = np.flatnonzero((tgt >= k * n_per_core) & (tgt < (k + 1) * n_per_core))
        loc = (tgt[sel] - k * n_per_core).astype(np.int64)
        gid = src_other[sel].astype(np.int64)
        b = loc // P
        c = gid // CHUNK
        order = np.lexsort((b, c, b // SB))
        loc = loc[order]
        gid = gid[order]
        np.add.at(counts[k], (b[order], c[order]), 1)
        cores.append((gid, loc))
    L = (np.ceil(counts.max(axis=0) / P) * P).astype(np.int64)  # [nblk, nchunk]
    empty = L.sum(axis=1) == 0
    L[empty, 0] = P
    return L, cores


def _col_of(L, nsb):
    """Column index per (b, c) in (sb, c, b) iteration order."""
    nblk, nchunk = L.shape
    col_of = np.zeros((nblk, nchunk), np.int64)
    col = 0
    for sb in range(nsb):
        blo, bhi = sb * SB, min((sb + 1) * SB, nblk)
        for c in range(nchunk):
            for b in range(blo, bhi):
                col_of[b, c] = col
                col += int(L[b, c]) // P
    return col_of, col


def _fill_dir(L, cores, nchunk, nsb, col_of, T_total):
    nblk = L.shape[0]
    gi = [np.zeros(T_total * P, np.int16) for _ in range(NCORES)]
    dr = [np.full(T_total * P, 999.0, np.float32) for _ in range(NCORES)]
    for k in range(NCORES):
        gid, loc = cores[k]
        b = loc // P
        c = gid // CHUNK
        key = ((b // SB) * nchunk + c) * nblk + b
        for bb in range(nblk):
            for cc in range(nchunk):
                if L[bb, cc] == 0:
                    continue
                kv = ((bb // SB) * nchunk + cc) * nblk + bb
                s = np.searchsorted(key, kv)
                e = np.searchsorted(key, kv, side="right")
                n = e - s
                assert n <= L[bb, cc]
                base = int(col_of[bb, cc]) * P
                gi[k][base:base + n] = (gid[s:e] - cc * CHUNK).astype(np.int16)
                dr[k][base:base + n] = (loc[s:e] - bb * P).astype(np.float32)
    dr = [a.reshape(T_total, P).T.copy() for a in dr]
    return gi, dr


def _call_list(L, nsb, col_of):
    """[(c, nidx, wofs, colstart)] per (sb, c) segment, split at MAXC."""
    nblk, nchunk = L.shape
    calls = []
    wofs = 0
    for sb in range(nsb):
        blo, bhi = sb * SB, min((sb + 1) * SB, nblk)
        for c in range(nchunk):
            seg = int(sum(L[b, c] for b in range(blo, bhi)))
            if seg == 0:
                continue
            col = int(col_of[blo, c]) if L[blo, c] > 0 else None
            # colstart = first col of the segment
            colstart = None
            for b in range(blo, bhi):
                if L[b, c] > 0:
                    colstart = int(col_of[b, c])
                    break
            for off in range(0, seg, MAXC):
                n = min(MAXC, seg - off)
                calls.append((c, n, wofs, colstart + off // P))
                wofs += n // 16
    return calls, wofs


def _wrap_idx(gi_flat, calls):
    bufs = []
    for (c, nidx, wofs, col) in calls:
        seg = gi_flat[col * P: col * P + nidx]
        w = seg.reshape(nidx // 16, 16).T
        bufs.append(np.tile(w, (8, 1)))
    return np.ascontiguousarray(np.concatenate(bufs, axis=1))


def _build_program(N, D, nblk, npad, nsb, Lin, Lout, calls_in, calls_out,
                   colof_in, colof_out, T_in, T_out, W_in, W_out):
    nc = bacc.Bacc("TRN2", target_bir_lowering=False, debug=False,
                   num_devices=NCORES, num_swdge_queues=NQ,
                   dynamic_dma_scratch_size=SCRATCH)
    f32 = mybir.dt.float32
    f16 = mybir.dt.float16
    nchunk = Lin.shape[1]
    xg_d = nc.dram_tensor("xg", [N, D], f32, kind="ExternalInput")
    xt_d = nc.dram_tensor("xt", [D, npad], f16, kind="ExternalInput")
    gii_d = nc.dram_tensor("gii", [P, W_in], mybir.dt.int16, kind="ExternalInput")
    gio_d = nc.dram_tensor("gio", [P, W_out], mybir.dt.int16, kind="ExternalInput")
    dri_d = nc.dram_tensor("dri", [P, T_in], f16, kind="ExternalInput")
    dro_d = nc.dram_tensor("dro", [P, T_out], f16, kind="ExternalInput")
    ivi_d = nc.dram_tensor("ivi", [P, nblk], f32, kind="ExternalInput")
    ivo_d = nc.dram_tensor("ivo", [P, nblk], f32, kind="ExternalInput")
    w1_d = nc.dram_tensor("w1", [D, D], f16, kind="ExternalInput")
    w2_d = nc.dram_tensor("w2", [D, D], f16, kind="ExternalInput")
    w3_d = nc.dram_tensor("w3", [D, D], f16, kind="ExternalInput")
    bb_d = nc.dram_tensor("bb", [1, D], f16, kind="ExternalInput")
    out_d = nc.dram_tensor("out", [npad, D], f32, kind="ExternalOutput")

    def seg_tiles(L, sb, c):
        blo, bhi = sb * SB, min((sb + 1) * SB, nblk)
        return int(sum(L[b, c] for b in range(blo, bhi))) // P

    maxseg = max(max(seg_tiles(Lin, sb, c) for sb in range(nsb)
                     for c in range(nchunk)),
                 max(seg_tiles(Lout, sb, c) for sb in range(nsb)
                     for c in range(nchunk)))

    with tile.TileContext(nc) as tc:
        with (
            tc.tile_pool(name="const", bufs=1) as cpool,
            tc.tile_pool(name="dest", bufs=5) as dpool,
            tc.tile_pool(name="mpool", bufs=6) as mpool,
            tc.tile_pool(name="hpool", bufs=6) as hpool,
            tc.tile_pool(name="fin", bufs=3) as fpool,
            tc.tile_pool(name="agg", bufs=2) as apool,
            tc.tile_pool(name="pbi", bufs=2, space="PSUM") as pbi,
            tc.tile_pool(name="pbo", bufs=2, space="PSUM") as pbo,
            tc.tile_pool(name="opp", bufs=2, space="PSUM") as opp,
            tc.tile_pool(name="app", bufs=1, space="PSUM") as app,
        ):
            nc.gpsimd.load_library(mlp)
            gii_sb = cpool.tile([P, W_in], mybir.dt.int16, name="gii_sb")
            nc.sync.dma_start(out=gii_sb[:], in_=gii_d[:])
            gio_sb = cpool.tile([P, W_out], mybir.dt.int16, name="gio_sb")
            nc.sync.dma_start(out=gio_sb[:], in_=gio_d[:])
            dri_sb = cpool.tile([P, T_in], f16, name="dri_sb")
            nc.sync.dma_start(out=dri_sb[:], in_=dri_d[:])
            dro_sb = cpool.tile([P, T_out], f16, name="dro_sb")
            nc.sync.dma_start(out=dro_sb[:], in_=dro_d[:])
            ivi_sb = cpool.tile([P, nblk], f32, name="ivi_sb")
            nc.sync.dma_start(out=ivi_sb[:], in_=ivi_d[:])
            ivo_sb = cpool.tile([P, nblk], f32, name="ivo_sb")
            nc.sync.dma_start(out=ivo_sb[:], in_=ivo_d[:])
            xt_sb = cpool.tile([D, npad], f16, name="xt_sb")
            nc.sync.dma_start(out=xt_sb[:], in_=xt_d[:])
            w1_sb = cpool.tile([D, D], f16, name="w1_sb")
            nc.sync.dma_start(out=w1_sb[:], in_=w1_d[:])
            w2_sb = cpool.tile([D, D], f16, name="w2_sb")
            nc.sync.dma_start(out=w2_sb[:], in_=w2_d[:])
            w3_sb = cpool.tile([D, D], f16, name="w3_sb")
            nc.sync.dma_start(out=w3_sb[:], in_=w3_d[:])
            bb_sb = cpool.tile([1, D], f16, name="bb_sb")
            nc.sync.dma_start(out=bb_sb[:], in_=bb_d[:])
            iota_t = cpool.tile([P, P], f16, name="iota_t")
            nc.gpsimd.iota(iota_t[:], pattern=[[1, P]], base=0,
                           channel_multiplier=0,
                           allow_small_or_imprecise_dtypes=True)
            ones1 = cpool.tile([1, P], f16, name="ones1")
            nc.vector.memset(ones1[:], 1.0)

            qcnt = {"q": 0}

            def do_sb(sb, dn, gi_sb, dr_sb, L, colof, calls, pool):
                blo, bhi = sb * SB, min((sb + 1) * SB, nblk)
                nb = bhi - blo
                pb = pool.tile([D, SB * P], f32, tag="pb", name=f"pb_{dn}{sb}")
                # first/last (c, t) per block for start/stop flags
                first_col = {}
                last_col = {}
                for b in range(blo, bhi):
                    cols = []
                    for c in range(nchunk):
                        if L[b, c] > 0:
                            s = int(colof[b, c])
                            cols += list(range(s, s + int(L[b, c]) // P))
                    first_col[b] = cols[0]
                    last_col[b] = cols[-1]
                m_tiles = {}
                h_tiles = {}
                segstarts = {}
                for c in range(nchunk):
                    ntg = seg_tiles(L, sb, c)
                    if ntg == 0:
                        continue
                    segstart = min(int(colof[b, c]) for b in range(blo, bhi)
                                   if L[b, c] > 0)
                    segstarts[c] = segstart
                    dest = dpool.tile([P, maxseg, D], f32, tag=f"dest_{dn}",
                                      name=f"dest_{dn}{sb}_{c}")
                    for (cc, nidx, wofs, colstart) in calls:
                        if cc != c or colstart < segstart or \
                                colstart >= segstart + ntg:
                            continue
                        rel = colstart - segstart
                        ncols = nidx // P
                        nc.gpsimd.dma_gather(
                            out_ap=dest[:, rel:rel + ncols, :],
                            in_ap=xg_d[c * CHUNK:min((c + 1) * CHUNK, N), :],
                            idxs_ap=gi_sb[:, wofs:wofs + nidx // 16],
                            num_idxs=nidx,
                            num_idxs_reg=nidx,
                            elem_size=D,
                            single_packet=False,
                            queue_num=qcnt["q"] % NQ,
                        )
                        qcnt["q"] += 1
                    m = mpool.tile([P, maxseg, D], f16, tag=f"m_{dn}",
                                   name=f"m_{dn}{sb}_{c}")
                    nc.scalar.copy(out=m[:, 0:ntg, :],
                                   in_=dest[:, 0:ntg, :])
                    h = hpool.tile([P, maxseg, P], f16, tag=f"h_{dn}",
                                   name=f"h_{dn}{sb}_{c}")
                    nc.vector.tensor_tensor(
                        out=h[:, 0:ntg, :],
                        in0=dr_sb[:, segstart:segstart + ntg, None]
                            .to_broadcast([P, ntg, P]),
                        in1=iota_t[:, None, :].to_broadcast([P, ntg, P]),
                        op=mybir.AluOpType.is_equal)
                    m_tiles[c] = m
                    h_tiles[c] = h
                # one PSUM accumulation group at a time per bank: finish
                # each block's chain before starting the next
                for b in range(blo, bhi):
                    j = b - blo
                    for c in range(nchunk):
                        if L[b, c] == 0:
                            continue
                        s = int(colof[b, c])
                        segstart = segstarts[c]
                        for t in range(int(L[b, c]) // P):
                            col = s + t
                            rel = col - segstart
                            nc.tensor.matmul(
                                pb[:, j * P:(j + 1) * P],
                                lhsT=m_tiles[c][:, rel, :],
                                rhs=h_tiles[c][:, rel, :],
                                start=(col == first_col[b]),
                                stop=(col == last_col[b]))
                aggTg = apool.tile([D, SB * P], f16, tag=f"aggTg_{dn}",
                                   name=f"aggTg_{dn}{sb}")
                nc.vector.tensor_copy(out=aggTg[:, 0:nb * P],
                                      in_=pb[:, 0:nb * P])
                return aggTg

            for sb in range(nsb):
                aggTg_in = do_sb(sb, "in", gii_sb, dri_sb, Lin, colof_in,
                                 calls_in, pbi)
                aggTg_out = do_sb(sb, "out", gio_sb, dro_sb, Lout, colof_out,
                                  calls_out, pbo)
                blo, bhi = sb * SB, min((sb + 1) * SB, nblk)
                for b in range(blo, bhi):
                    j = b - blo
                    ops = opp.tile([P, D], f32, tag="ops", name=f"ops{b}")
                    nc.tensor.matmul(ops[:], lhsT=xt_sb[:, b * P:(b + 1) * P],
                                     rhs=w1_sb[:], start=True, stop=False)
                    nc.tensor.matmul(ops[:], lhsT=ones1[:], rhs=bb_sb[:],
                                     start=False, stop=True)
                    aps_in = app.tile([P, D], f32, tag="apsi", name=f"apsi{b}")
                    nc.tensor.matmul(aps_in[:],
                                     lhsT=aggTg_in[:, j * P:(j + 1) * P],
                                     rhs=w2_sb[:], start=True, stop=True)
                    aps_out = app.tile([P, D], f32, tag="apso",
                                       name=f"apso{b}")
                    nc.tensor.matmul(aps_out[:],
                                     lhsT=aggTg_out[:, j * P:(j + 1) * P],
                                     rhs=w3_sb[:], start=True, stop=True)
                    c1 = fpool.tile([P, D], f32, tag="c1", name=f"c1{b}")
                    nc.vector.tensor_tensor(
                        out=c1[:], in0=aps_in[:],
                        in1=ivi_sb[:, b:b + 1].to_broadcast([P, D]),
                        op=mybir.AluOpType.mult)
                    c2 = fpool.tile([P, D], f32, tag="c2", name=f"c2{b}")
                    nc.vector.tensor_tensor(
                        out=c2[:], in0=aps_out[:],
                        in1=ivo_sb[:, b:b + 1].to_broadcast([P, D]),
                        op=mybir.AluOpType.mult)
                    nc.vector.tensor_tensor(out=c1[:], in0=c1[:], in1=c2[:],
                                            op=mybir.AluOpType.add)
                    osb = fpool.tile([P, D], f32, tag="osb", name=f"osb{b}")
                    nc.vector.tensor_tensor(out=osb[:], in0=c1[:], in1=ops[:],
                                            op=mybir.AluOpType.add)
                    nc.sync.dma_start(out=out_d[b * P:(b + 1) * P, :],
                                      in_=osb[:])

    nc.compile()
    return nc


def kernel(x, edge_index, W_s2d, W_d2s, W_self, b_self):
    global last_exec_time_ns
    import ml_dtypes
    f16 = np.float16
    N, D = x.shape
    assert N % NCORES == 0
    n_per_core = N // NCORES
    nblk = (n_per_core + P - 1) // P
    npad = nblk * P
    nsb = (nblk + SB - 1) // SB
    nchunk = (N + CHUNK - 1) // CHUNK
    src = np.asarray(edge_index[0], dtype=np.int64)
    dst = np.asarray(edge_index[1], dtype=np.int64)

    Lin, cores_in = _schedule(dst, src, n_per_core, nblk, nchunk)
    Lout, cores_out = _schedule(src, dst, n_per_core, nblk, nchunk)
    colof_in, T_in = _col_of(Lin, nsb)
    colof_out, T_out = _col_of(Lout, nsb)
    gi_in, dr_in = _fill_dir(Lin, cores_in, nchunk, nsb, colof_in, T_in)
    gi_out, dr_out = _fill_dir(Lout, cores_out, nchunk, nsb, colof_out, T_out)
    calls_in, W_in = _call_list(Lin, nsb, colof_in)
    calls_out, W_out = _call_list(Lout, nsb, colof_out)
    gii_list = [_wrap_idx(gi_in[k], calls_in) for k in range(NCORES)]
    gio_list = [_wrap_idx(gi_out[k], calls_out) for k in range(NCORES)]

    # host-side inverse degrees per local node, laid out [P, nblk]
    def inv_deg(tgt):
        cnt = np.bincount(tgt, minlength=N).astype(np.float64)
        iv = (1.0 / np.maximum(cnt, 1.0)).astype(np.float32)
        out = np.zeros((NCORES, npad), np.float32)
        for k in range(NCORES):
            out[k, :n_per_core] = iv[k * n_per_core:(k + 1) * n_per_core]
        return [np.ascontiguousarray(out[k].reshape(nblk, P).T)
                for k in range(NCORES)]

    ivi_list = inv_deg(dst)
    ivo_list = inv_deg(src)

    x32 = np.ascontiguousarray(x, dtype=np.float32)
    xt = np.zeros((NCORES, D, npad), f16)
    for k in range(NCORES):
        xt[k, :, :n_per_core] = \
            x32[k * n_per_core:(k + 1) * n_per_core].T.astype(f16)

    w1 = np.ascontiguousarray(np.asarray(W_self, np.float32).astype(f16))
    w2 = np.ascontiguousarray(
        ((1.0 - ALPHA) * np.asarray(W_s2d, np.float32)).astype(f16))
    w3 = np.ascontiguousarray(
        (ALPHA * np.asarray(W_d2s, np.float32)).astype(f16))
    bb = np.asarray(b_self, np.float32).reshape(1, D).astype(f16)

    nc = _build_program(N, D, nblk, npad, nsb, Lin, Lout, calls_in, calls_out,
                        colof_in, colof_out, T_in, T_out, W_in, W_out)
    in_maps = [
        {"xg": x32, "xt": np.ascontiguousarray(xt[k]),
         "gii": gii_list[k], "gio": gio_list[k],
         "dri": dr_in[k].astype(f16), "dro": dr_out[k].astype(f16),
         "ivi": ivi_list[k], "ivo": ivo_list[k],
         "w1": w1, "w2": w2, "w3": w3, "bb": bb}
        for k in range(NCORES)
    ]
    res = run_bass_kernel_spmd(nc, in_maps, list(range(NCORES)),
                               trace=_tracing_enabled())
    last_exec_time_ns = res.exec_time_ns
    out = np.concatenate(
        [res.results[k]["out"][:n_per_core] for k in range(NCORES)], axis=0)
    return out


def _tracing_enabled():
    import os
    return os.environ.get("KERNEL_TRACE", "0") == "1"


if True:
    try:
        import contextlib
        import ctypes
        import sys
        import types

        def _mk():
            lib = ctypes.CDLL("/opt/axon/libaxon_pjrt.so")
            if not hasattr(lib, "axon_start_nrt_profile"):
                return None
            lib.axon_start_nrt_profile.argtypes = [
                ctypes.POINTER(ctypes.c_int64), ctypes.c_size_t]
            lib.axon_start_nrt_profile.restype = ctypes.c_int64
            lib.axon_stop_nrt_profile.argtypes = [ctypes.c_char_p]
            lib.axon_stop_nrt_profile.restype = ctypes.c_int64

            @contextlib.contextmanager
            def _hook(output_dir, device_ids):
                import jax
                jax.devices()
                if device_ids:
                    ids = (ctypes.c_int64 * len(device_ids))(*device_ids)
                    rc = lib.axon_start_nrt_profile(ids, len(device_ids))
                else:
                    rc = lib.axon_start_nrt_profile(None, 0)
                if rc != 0:
                    raise RuntimeError(f"start_nrt_profile rc={rc}")
                try:
                    yield
                finally:
                    n = lib.axon_stop_nrt_profile(str(output_dir).encode())
                    if n < 0:
                        raise RuntimeError(f"stop_nrt_profile rc={n}")

            return _hook

        _hk = _mk()
        _m = types.ModuleType("antenv.axon_hooks")
        _m.get_axon_ntff_profile_hook = lambda: _hk
        sys.modules["antenv.axon_hooks"] = _m
        import concourse.bass_utils as _bu
        _bu.upload_artifacts = lambda t: str(t)
    except Exception:
        pass
